# revision 6
# baseline (speedup 1.0000x reference)
"""Causal self-attention with RoPE on 8 trn2 NeuronCores.

Sharding: tensor-parallel over heads (Megatron style). 16 heads, 8 cores
-> 2 heads per core. Each core computes q/k/v for its 2 heads, causal
attention, and a partial output projection against its w_o column slice.
Host sums the 8 partial outputs (bf16 partials; the Megatron all-reduce
is done at gather).

v2 design: keep the PE (tensor engine) streaming continuously. TRN2's PE
p-state throttles to 1.2 GHz for ~3us after any idle gap, so every stall
costs double. Mechanisms:
 - All psum accumulators are [128,512] single-bank tiles in one 6-slot
   rotation (tag "rot"): qe/qo/ke/ko/va/vb per QKV j-block, then score
   tiles / transpose tiles / wo tiles in attention. A slot's drain always
   finishes a full phase before the slot comes around again.
 - QKV j-loop: v-projection matmuls lag the q/k matmuls by 2 c-steps so
   the previous block's psum drains (split across ACT and DVE) complete
   before their slots are reused. Rope runs in bf16 on DVE (2-4x modes).
 - Attention i-loop is software-pipelined: score matmul for tile i+1
   issues before the AV matmuls of tile i, so the PE never waits on the
   Scalar-engine exp. Exp is column-restricted near the diagonal; causal
   masking is a single [128,128] upper-tri multiply on the one partially
   -valid 128-col block. AV matmuls accumulate y and the softmax
   denominator together via a ones-column in v (129-wide rhs).
 - The output projection of block j is chopped into micro-ops sprinkled
   into block j+1's i-loop as PE filler; drains alternate DVE/Pool; the
   partial output is stored bf16 (halves HBM writes).
 - Weights are host-pre-arranged to contiguous [128, X] layouts; w_qkv
   loads in 4 chunks so the first matmul starts ~7us in.
"""

import math

import numpy as np

B, T, C, H = 2, 2048, 2048, 16
D = C // H  # 128
NCORES = 8
HPC = H // NCORES  # heads per core = 2
N = B * T  # 4096 token rows
TT = T // 128  # 16 t-tiles per batch
NJ = T // 512  # 4 j blocks of 512 per batch
CT = C // 128  # 16 contraction tiles
VW = D + 2  # 130 per (tile, head): [v(128) | 1 | pad]
TW = HPC * VW  # 260 per t-tile

_COMPILED = None


def _build():
    import concourse.bacc as bacc
    import concourse.mybir as mybir
    import concourse.tile as tile
    from concourse.masks import make_identity

    f32 = mybir.dt.float32
    bf16 = mybir.dt.bfloat16

    nc = bacc.Bacc("TRN2", target_bir_lowering=False, debug=False)
    xT = nc.declare_dram_parameter("xT", [C, N], bf16, isOutput=False)
    wqk2 = nc.declare_dram_parameter("wqk2", [128, CT * 512], bf16, isOutput=False)
    wv2 = nc.declare_dram_parameter("wv2", [128, CT * 256], bf16, isOutput=False)
    wo2 = nc.declare_dram_parameter("wo2", [128, HPC * C], bf16, isOutput=False)
    cos2 = nc.declare_dram_parameter("cos2", [128, N], bf16, isOutput=False)
    sin2 = nc.declare_dram_parameter("sin2", [128, N], bf16, isOutput=False)
    tri = nc.declare_dram_parameter("tri", [128, 128], bf16, isOutput=False)
    out_p = nc.declare_dram_parameter("out_p", [N, C], bf16, isOutput=True)

    SCALE = 1.0 / math.sqrt(D)
    EXPF = mybir.ActivationFunctionType.Exp

    with tile.TileContext(nc) as tc:
        with (
            tc.tile_pool(name="wpool", bufs=1) as wpool,
            tc.tile_pool(name="xpool", bufs=6) as xpool,
            tc.tile_pool(name="pcp", bufs=8) as pcpool,
            tc.tile_pool(name="rtp", bufs=8) as rtpool,
            tc.tile_pool(name="rop", bufs=8) as ropool,
            tc.tile_pool(name="qkh", bufs=4) as qkhpool,
            tc.tile_pool(name="vsb", bufs=1) as vpool,
            tc.tile_pool(name="exp", bufs=8) as expool,
            tc.tile_pool(name="ysb", bufs=4) as ysbpool,
            tc.tile_pool(name="rcp", bufs=4) as rpool,
            tc.tile_pool(name="ytp", bufs=4) as ytpool,
            tc.tile_pool(name="yop", bufs=4) as yopool,
            tc.tile_pool(name="prot", bufs=6, space="PSUM") as prot,
            tc.tile_pool(name="pacc", bufs=2, space="PSUM") as pacc,
        ):
            # ---- resident weights / constants ----
            wqk_sb = wpool.tile([128, CT * 512], bf16, tag="wqk")
            for ch in range(4):
                cs = slice(ch * CT * 128, (ch + 1) * CT * 128)
                nc.sync.dma_start(out=wqk_sb[:, cs], in_=wqk2[:, cs])
            wv_sb = wpool.tile([128, CT * 256], bf16, tag="wv")
            nc.sync.dma_start(out=wv_sb[:, :], in_=wv2[:, :])
            cos_sb = wpool.tile([128, N], bf16, tag="cos")
            nc.sync.dma_start(out=cos_sb[:, :], in_=cos2[:, :])
            sin_sb = wpool.tile([128, N], bf16, tag="sin")
            nc.sync.dma_start(out=sin_sb[:, :], in_=sin2[:, :])
            tri_sb = wpool.tile([128, 128], bf16, tag="tri")
            nc.sync.dma_start(out=tri_sb[:, :], in_=tri[:, :])
            wo_sb = wpool.tile([128, HPC * C], bf16, tag="wo")
            nc.sync.dma_start(out=wo_sb[:, :], in_=wo2[:, :])
            ident = wpool.tile([128, 128], bf16, tag="ident")
            make_identity(nc, ident[:, :])

            v_sb = vpool.tile([128, TT * TW], bf16, tag="vsb")
            for tt in range(TT):
                for h in range(HPC):
                    col = tt * TW + h * VW + 128
                    nc.vector.memset(v_sb[:, col : col + 1], 1.0)

            # cross-batch xt prefetch stash: (b, j, c) -> tile
            stash = {}

            def xt_dma(b, j, c):
                t = xpool.tile([128, 512], bf16, tag="xt", name="xt")
                nc.gpsimd.dma_start(
                    out=t[:, :],
                    in_=xT[c * 128 : (c + 1) * 128, b * T + j * 512 : b * T + (j + 1) * 512],
                )
                return t

            def get_xt(b, j, c):
                return stash.pop((b, j, c), None) or xt_dma(b, j, c)

            for b in range(B):
                n0 = b * T
                qhat = [qkhpool.tile([128, T], bf16, tag="qh", name=f"qhat{_h}") for _h in range(HPC)]
                khat = [qkhpool.tile([128, T], bf16, tag="qh", name=f"khat{_h}") for _h in range(HPC)]

                # ================= QKV phase =================
                for j in range(NJ):
                    qe = prot.tile([128, 512], f32, tag="rot", name="qe")
                    qo = prot.tile([128, 512], f32, tag="rot", name="qo")
                    ke = prot.tile([128, 512], f32, tag="rot", name="ke")
                    ko = prot.tile([128, 512], f32, tag="rot", name="ko")
                    va = prot.tile([128, 512], f32, tag="rot", name="va")
                    vb = prot.tile([128, 512], f32, tag="rot", name="vb")
                    qk_dst = (qe, qo, ke, ko)
                    xts = {}

                    def vmm(cv):
                        # two 256-col accumulators share each psum bank:
                        # only the first-issued one zeroes the bank (start),
                        # only the last-finishing one closes it (stop).
                        xv = xts.pop(cv)
                        for tl in range(4):
                            dst = va if tl < 2 else vb
                            o = (tl % 2) * 256
                            nc.tensor.matmul(
                                dst[:, o : o + 256],
                                xv[:, tl * 128 : (tl + 1) * 128],
                                wv_sb[:, cv * 256 : (cv + 1) * 256],
                                start=(cv == 0 and tl % 2 == 0),
                                stop=(cv == CT - 1 and tl % 2 == 1),
                            )

                    for c in range(CT):
                        xt = get_xt(b, j, c)
                        xts[c] = xt
                        for part in range(4):
                            nc.tensor.matmul(
                                qk_dst[part][:, :],
                                wqk_sb[:, c * 512 + part * 128 : c * 512 + (part + 1) * 128],
                                xt[:, :],
                                start=(c == 0),
                                stop=(c == CT - 1),
                            )
                        if c >= 2:
                            vmm(c - 2)
                    # prefetch next block's first x tiles before drains queue up
                    if j + 1 < NJ:
                        for c in (0, 1):
                            stash[(b, j + 1, c)] = xt_dma(b, j + 1, c)
                    vmm(CT - 2)
                    vmm(CT - 1)

                    # drains: ACT gets qe,qo,vb; DVE gets ke,ko,va
                    pc_qe = pcpool.tile([128, 512], bf16, tag="pc", name="pc_qe")
                    pc_qo = pcpool.tile([128, 512], bf16, tag="pc", name="pc_qo")
                    pc_ke = pcpool.tile([128, 512], bf16, tag="pc", name="pc_ke")
                    pc_ko = pcpool.tile([128, 512], bf16, tag="pc", name="pc_ko")
                    nc.scalar.copy(pc_qe[:, :], qe[:, :])
                    nc.vector.tensor_copy(pc_ke[:, :], ke[:, :])
                    nc.scalar.copy(pc_qo[:, :], qo[:, :])
                    nc.vector.tensor_copy(pc_ko[:, :], ko[:, :])
                    base = j * 4 * TW
                    for half, (src, eng) in enumerate(((va, nc.vector), (vb, nc.scalar))):
                        dst = (
                            v_sb[:, base + half * 2 * TW : base + (half + 1) * 2 * TW]
                            .rearrange("p (t h x) -> p t h x", t=2, h=HPC)[:, :, :, 0:128]
                        )
                        s = src[:, :].rearrange("p (t h x) -> p t h x", t=2, h=HPC)
                        if eng is nc.scalar:
                            nc.scalar.copy(dst, s)
                        else:
                            nc.vector.tensor_copy(dst, s)

                    # rope (bf16, DVE)
                    ce = cos_sb[:, n0 + j * 512 : n0 + (j + 1) * 512]
                    se = sin_sb[:, n0 + j * 512 : n0 + (j + 1) * 512]
                    outs = []
                    for name, pe_, po_ in (("q", pc_qe, pc_qo), ("k", pc_ke, pc_ko)):
                        oe = ropool.tile([128, 512], bf16, tag="ro", name=f"{name}e_r")
                        oo = ropool.tile([128, 512], bf16, tag="ro", name=f"{name}o_r")
                        t1 = rtpool.tile([128, 512], bf16, tag="rt", name="t1")
                        t2 = rtpool.tile([128, 512], bf16, tag="rt", name="t2")
                        nc.vector.tensor_mul(t1[:, :], pe_[:, :], ce)
                        nc.vector.tensor_mul(t2[:, :], po_[:, :], se)
                        nc.vector.tensor_sub(oe[:, :], t1[:, :], t2[:, :])
                        t3 = rtpool.tile([128, 512], bf16, tag="rt", name="t3")
                        t4 = rtpool.tile([128, 512], bf16, tag="rt", name="t4")
                        nc.vector.tensor_mul(t3[:, :], pe_[:, :], se)
                        nc.vector.tensor_mul(t4[:, :], po_[:, :], ce)
                        nc.vector.tensor_add(oo[:, :], t3[:, :], t4[:, :])
                        outs.append((oe, oo))
                    js = slice(j * 512, (j + 1) * 512)
                    for h in range(HPC):
                        hs = slice(64 * h, 64 * h + 64)
                        nc.sync.dma_start(out=qhat[h][0:64, js], in_=outs[0][0][hs, :])
                        nc.sync.dma_start(out=qhat[h][64:128, js], in_=outs[0][1][hs, :])
                        nc.sync.dma_start(out=khat[h][0:64, js], in_=outs[1][0][hs, :])
                        nc.sync.dma_start(out=khat[h][64:128, js], in_=outs[1][1][hs, :])

                # ================= attention + wo phase =================
                yT = [ytpool.tile([128, T], bf16, tag="yt", name=f"yT{_h}") for _h in range(HPC)]
                wo_items = []  # pending (tt, ob, yo, state) micro-ops
                drain_flip = [0]

                def emit_wo(k=1):
                    for _ in range(k):
                        if not wo_items:
                            return
                        tt, ob, yo, state = wo_items.pop(0)
                        o_ps = prot.tile([128, 512], f32, tag="rot", name="o_ps")
                        for h in range(HPC):
                            nc.tensor.matmul(
                                o_ps[:, :],
                                yT[h][:, tt * 128 : (tt + 1) * 128],
                                wo_sb[:, h * C + ob * 512 : h * C + (ob + 1) * 512],
                                start=(h == 0),
                                stop=(h == HPC - 1),
                            )
                        if drain_flip[0] % 2 == 0:
                            nc.vector.tensor_copy(yo[:, ob * 512 : (ob + 1) * 512], o_ps[:, :])
                        else:
                            nc.scalar.copy(yo[:, ob * 512 : (ob + 1) * 512], o_ps[:, :])
                        drain_flip[0] += 1
                        state[0] += 1
                        if state[0] == 4:
                            nc.sync.dma_start(
                                out=out_p[n0 + tt * 128 : n0 + (tt + 1) * 128, :],
                                in_=yo[:, :],
                            )

                for j in range(NJ):
                    for h in range(HPC):
                        y_a = pacc.tile([128, 512], f32, tag="acc", name="y_a")
                        y_b = pacc.tile([128, 512], f32, tag="acc", name="y_b")
                        nsc = 4 * j + 4
                        sc_tiles = {}

                        def issue_sc(i):
                            p = i - 4 * j
                            lo = max(0, p) * 128
                            sc = prot.tile([128, 512], f32, tag="rot", name="sc")
                            nc.tensor.matmul(
                                sc[:, lo:512],
                                khat[h][:, i * 128 : (i + 1) * 128],
                                qhat[h][:, j * 512 + lo : (j + 1) * 512],
                                start=True,
                                stop=True,
                            )
                            sc_tiles[i] = sc

                        issue_sc(0)
                        for i in range(nsc):
                            p = i - 4 * j
                            lo = max(0, p) * 128
                            sc = sc_tiles.pop(i)
                            ex = expool.tile([128, 512], bf16, tag="ex", name="ex")
                            nc.scalar.activation(
                                ex[:, lo:512], sc[:, lo:512], EXPF, scale=SCALE
                            )
                            if p >= 0:
                                nc.gpsimd.tensor_mul(
                                    ex[:, lo : lo + 128], ex[:, lo : lo + 128], tri_sb[:, :]
                                )
                            if i + 1 < nsc:
                                issue_sc(i + 1)
                            emit_wo(2)
                            for tau in range(3, max(0, p) - 1, -1):
                                # tau pairs share a psum bank; the odd tau is
                                # issued first (descending loop) and finishes
                                # last, so it owns start and stop.
                                dst = y_a if tau < 2 else y_b
                                off = (tau % 2) * 256
                                hi = tau % 2 == 1
                                nc.tensor.matmul(
                                    dst[:, off : off + 129],
                                    ex[:, tau * 128 : (tau + 1) * 128],
                                    v_sb[:, i * TW + h * VW : i * TW + h * VW + 129],
                                    start=(i == 0 and hi),
                                    stop=(i == 4 * j + tau and hi),
                                )
                        for tau in range(4):
                            dst = y_a if tau < 2 else y_b
                            off = (tau % 2) * 256
                            r = rpool.tile([128, 1], f32, tag="r", name="r")
                            nc.vector.reciprocal(r[:, :], dst[:, off + 128 : off + 129])
                            y_sb = ysbpool.tile([128, 128], bf16, tag="y", name="y_sb")
                            nc.vector.tensor_scalar_mul(
                                y_sb[:, :], dst[:, off : off + 128], r[:, 0:1]
                            )
                            yt_ps = prot.tile([128, 128], bf16, tag="rot", name="yt_ps")
                            nc.tensor.transpose(yt_ps[:, :], y_sb[:, :], ident[:, :])
                            g = 4 * j + tau
                            nc.vector.tensor_copy(
                                yT[h][:, g * 128 : (g + 1) * 128], yt_ps[:, :]
                            )
                            emit_wo(1)
                    # enqueue wo micro-ops for this j (sprinkled into j+1)
                    for tt in range(4 * j, 4 * j + 4):
                        yo = yopool.tile([128, C], bf16, tag="yo", name="yo")
                        state = [0]
                        for ob in range(4):
                            wo_items.append((tt, ob, yo, state))
                # batch tail: prefetch next batch's x, then flush remaining wo
                if b + 1 < B:
                    for c in (0, 1):
                        stash[(b + 1, 0, c)] = xt_dma(b + 1, 0, c)
                emit_wo(len(wo_items))
    nc.finalize()
    return nc


def _prep_inputs(x, w_qkv, w_o, rope_cos, rope_sin):
    import ml_dtypes

    bf = ml_dtypes.bfloat16
    xTh = np.ascontiguousarray(x.reshape(N, C).T).astype(bf)
    cosT = np.ascontiguousarray(rope_cos.T)  # [64, T]
    sinT = np.ascontiguousarray(rope_sin.T)
    cos2 = np.tile(np.concatenate([cosT, cosT], 0), (1, B)).astype(bf)
    sin2 = np.tile(np.concatenate([sinT, sinT], 0), (1, B)).astype(bf)

    r = np.arange(128)[:, None]
    c = np.arange(128)[None, :]
    tri = (c >= r).astype(np.float32).astype(bf)

    ev = np.arange(0, D, 2)
    od = np.arange(1, D, 2)
    in_maps = []
    for m in range(NCORES):
        h0, h1 = 2 * m, 2 * m + 1
        # blocks QE|QO|KE|KO; within each, cols = [head0 dims | head1 dims]
        QE = np.concatenate([w_qkv[h0 * D + ev, :], w_qkv[h1 * D + ev, :]], 0).T
        QO = np.concatenate([w_qkv[h0 * D + od, :], w_qkv[h1 * D + od, :]], 0).T
        KE = np.concatenate([w_qkv[C + h0 * D + ev, :], w_qkv[C + h1 * D + ev, :]], 0).T
        KO = np.concatenate([w_qkv[C + h0 * D + od, :], w_qkv[C + h1 * D + od, :]], 0).T
        Wqk = np.concatenate([QE, QO, KE, KO], 1)  # [C, 512]
        wqk2 = Wqk.reshape(CT, 128, 512).transpose(1, 0, 2).reshape(128, CT * 512)
        Wv = w_qkv[2 * C + 2 * m * D : 2 * C + (2 * m + 2) * D, :].T  # [C, 256]
        wv2 = Wv.reshape(CT, 128, 256).transpose(1, 0, 2).reshape(128, CT * 256)
        Wo = w_o[:, 2 * m * D : (2 * m + 2) * D].T  # [256, C]
        wo2 = Wo.reshape(HPC, 128, C).transpose(1, 0, 2).reshape(128, HPC * C)
        in_maps.append(
            {
                "xT": xTh,
                "wqk2": np.ascontiguousarray(wqk2).astype(bf),
                "wv2": np.ascontiguousarray(wv2).astype(bf),
                "wo2": np.ascontiguousarray(wo2).astype(bf),
                "cos2": cos2,
                "sin2": sin2,
                "tri": np.ascontiguousarray(tri),
            }
        )
    return in_maps


def kernel(x, w_qkv, w_o, rope_cos, rope_sin, _trace=False):
    global _COMPILED
    x = np.asarray(x, dtype=np.float32)
    w_qkv = np.asarray(w_qkv, dtype=np.float32)
    w_o = np.asarray(w_o, dtype=np.float32)
    rope_cos = np.asarray(rope_cos, dtype=np.float32)
    rope_sin = np.asarray(rope_sin, dtype=np.float32)

    from concourse.bass_utils import run_bass_kernel_spmd

    if _COMPILED is None:
        _COMPILED = _build()
    nc = _COMPILED
    in_maps = _prep_inputs(x, w_qkv, w_o, rope_cos, rope_sin)
    res = run_bass_kernel_spmd(
        nc, in_maps, core_ids=list(range(NCORES)), trace=_trace
    )
    out = np.zeros((N, C), dtype=np.float32)
    for m in range(NCORES):
        out += res.results[m]["out_p"].astype(np.float32)
    kernel._last_results = res
    return out.reshape(B, T, C)


# revision 13
# speedup vs baseline: 1.2693x; 1.2693x over previous
"""Causal self-attention with RoPE on 8 trn2 NeuronCores.

Sharding: tensor-parallel over heads (Megatron style). 16 heads, 8 cores
-> 2 heads per core. Each core computes q/k/v for its 2 heads, causal
attention, and a partial output projection against its w_o column slice.
Host sums the 8 partial bf16 outputs (the Megatron all-reduce at gather).

v3 design: keep the PE streaming continuously at full clock. TRN2's PE
p-state drops to 1.2 GHz for ~3us after ANY idle gap, so the schedule is
built so the PE is the pacing engine everywhere:
 - QKV phase: three [128,1024] psum tiles per 512-token block j
   (q=[QE|QO], k=[KE|KO], v=[4 x 256]) rotate through a 3-slot pool.
   V matmuls lag q/k by two c-steps so the previous block's drains (one
   wide ACT copy for q, DVE for k, one strided DVE copy for v) finish
   before slots recycle. Rope uses a fused [cos|sin] layout: 2 muls + 1
   add/sub per q/k on DVE in bf16.
 - Attention runs on paired q-blocks (jlo,jhi) sharing each k-tile's
   score psum [128, jlo|jhi]: one wide exp per tile (amortizes the
   ~200ns ACT fixed cost). AV for jlo interleaves into the i-loop
   (pipelined one step behind the score matmul); AV for jhi is DEFERRED
   into a dependency-free all-PE sweep afterwards, which also lets the
   Scalar engine catch up. Exp is column-restricted at the diagonal;
   masking is one [128,128] upper-tri multiply on the Pool engine.
 - Output projection of pair jp is chopped into [128,1024] psum
   micro-ops used as PE filler inside the next pair's i-loop (issued
   between the score matmul and the exp-dependent AV matmuls).
 - Weights host-pre-arranged contiguous; wqk loads in 4 chunks so the
   first matmul starts early. Partial output stored bf16.
"""

import math

import numpy as np

B, T, C, H = 2, 2048, 2048, 16
D = C // H  # 128
NCORES = 8
HPC = H // NCORES  # heads per core = 2
N = B * T  # 4096 token rows
TT = T // 128  # 16 t-tiles per batch
NJ = T // 512  # 4 j blocks of 512 per batch
CT = C // 128  # 16 contraction tiles
VW = D + 2  # 130 per (tile, head): [v(128) | 1 | pad]
TW = HPC * VW  # 260 per t-tile

_COMPILED = None


def _build():
    import concourse.bacc as bacc
    import concourse.mybir as mybir
    import concourse.tile as tile
    from concourse.masks import make_identity

    f32 = mybir.dt.float32
    bf16 = mybir.dt.bfloat16

    nc = bacc.Bacc("TRN2", target_bir_lowering=False, debug=False)
    xT = nc.declare_dram_parameter("xT", [C, N], bf16, isOutput=False)
    wqk2 = nc.declare_dram_parameter("wqk2", [128, CT * 512], bf16, isOutput=False)
    wv2 = nc.declare_dram_parameter("wv2", [128, CT * 256], bf16, isOutput=False)
    wo2 = nc.declare_dram_parameter("wo2", [128, HPC * C], bf16, isOutput=False)
    cossin = nc.declare_dram_parameter("cossin", [128, 2 * N], bf16, isOutput=False)
    sincos = nc.declare_dram_parameter("sincos", [128, 2 * N], bf16, isOutput=False)
    tri = nc.declare_dram_parameter("tri", [128, 128], bf16, isOutput=False)
    out_p = nc.declare_dram_parameter("out_p", [N, C], bf16, isOutput=True)

    SCALE = 1.0 / math.sqrt(D)
    EXPF = mybir.ActivationFunctionType.Exp

    with tile.TileContext(nc) as tc:
        with (
            tc.tile_pool(name="wpool", bufs=1) as wpool,
            tc.tile_pool(name="xpool", bufs=6) as xpool,
            tc.tile_pool(name="pcp", bufs=4) as pcpool,
            tc.tile_pool(name="rtp", bufs=4) as rtpool,
            tc.tile_pool(name="rop", bufs=8) as ropool,
            tc.tile_pool(name="qkh", bufs=4) as qkhpool,
            tc.tile_pool(name="vsb", bufs=1) as vpool,
            tc.tile_pool(name="exp", bufs=20) as expool,
            tc.tile_pool(name="ysb", bufs=4) as ysbpool,
            tc.tile_pool(name="rcp", bufs=4) as rpool,
            tc.tile_pool(name="ytp", bufs=4) as ytpool,
            tc.tile_pool(name="yop", bufs=4) as yopool,
            tc.tile_pool(name="pbig", bufs=3, space="PSUM") as pbig,
            tc.tile_pool(name="pacc", bufs=2, space="PSUM") as pacc,
        ):
            # ---- resident weights / constants ----
            wqk_sb = wpool.tile([128, CT * 512], bf16, tag="wqk")
            for ch in range(4):
                cs = slice(ch * CT * 128, (ch + 1) * CT * 128)
                nc.sync.dma_start(out=wqk_sb[:, cs], in_=wqk2[:, cs])
            wv_sb = wpool.tile([128, CT * 256], bf16, tag="wv")
            nc.sync.dma_start(out=wv_sb[:, :], in_=wv2[:, :])
            cossin_sb = wpool.tile([128, 2 * N], bf16, tag="cossin")
            nc.sync.dma_start(out=cossin_sb[:, :], in_=cossin[:, :])
            sincos_sb = wpool.tile([128, 2 * N], bf16, tag="sincos")
            nc.sync.dma_start(out=sincos_sb[:, :], in_=sincos[:, :])
            tri_sb = wpool.tile([128, 128], bf16, tag="tri")
            nc.sync.dma_start(out=tri_sb[:, :], in_=tri[:, :])
            wo_sb = wpool.tile([128, HPC * C], bf16, tag="wo")
            nc.sync.dma_start(out=wo_sb[:, :], in_=wo2[:, :])
            ident = wpool.tile([128, 128], bf16, tag="ident")
            make_identity(nc, ident[:, :])

            v_sb = vpool.tile([128, TT * TW], bf16, tag="vsb")
            for tt in range(TT):
                for h in range(HPC):
                    col = tt * TW + h * VW + 128
                    nc.vector.memset(v_sb[:, col : col + 1], 1.0)

            stash = {}
            wo_items = []  # [tt, obp, yo, state, yT_list, n0]
            drain_flip = [0]

            def emit_wo(k=1):
                for _ in range(k):
                    if not wo_items:
                        return
                    tt, obp, yo, state, yTl, nn0 = wo_items.pop(0)
                    o_ps = pbig.tile([128, 1024], f32, tag="big", name="o_ps")
                    for ob in (2 * obp, 2 * obp + 1):
                        o = (ob % 2) * 512
                        for h2 in range(HPC):
                            nc.tensor.matmul(
                                o_ps[:, o : o + 512],
                                yTl[h2][:, tt * 128 : (tt + 1) * 128],
                                wo_sb[:, h2 * C + ob * 512 : h2 * C + (ob + 1) * 512],
                                start=(h2 == 0),
                                stop=(h2 == HPC - 1),
                            )
                    if drain_flip[0] % 2 == 0:
                        nc.vector.tensor_copy(yo[:, obp * 1024 : (obp + 1) * 1024], o_ps[:, :])
                    else:
                        nc.scalar.copy(yo[:, obp * 1024 : (obp + 1) * 1024], o_ps[:, :])
                    drain_flip[0] += 1
                    state[0] += 1
                    if state[0] == 2:
                        nc.sync.dma_start(
                            out=out_p[nn0 + tt * 128 : nn0 + (tt + 1) * 128, :],
                            in_=yo[:, :],
                        )

            def xt_dma(b, j, c):
                t = xpool.tile([128, 512], bf16, tag="xt", name="xt")
                nc.gpsimd.dma_start(
                    out=t[:, :],
                    in_=xT[c * 128 : (c + 1) * 128, b * T + j * 512 : b * T + (j + 1) * 512],
                )
                return t

            def get_xt(b, j, c):
                t = stash.pop((b, j, c), None)
                return t if t is not None else xt_dma(b, j, c)

            for b in range(B):
                n0 = b * T
                qhat = [qkhpool.tile([128, T], bf16, tag="qh", name=f"qhat{_h}") for _h in range(HPC)]
                khat = [qkhpool.tile([128, T], bf16, tag="qh", name=f"khat{_h}") for _h in range(HPC)]

                # ================= QKV phase =================
                for j in range(NJ):
                    ps_q = pbig.tile([128, 1024], f32, tag="big", name="ps_q")
                    ps_k = pbig.tile([128, 1024], f32, tag="big", name="ps_k")
                    ps_v = pbig.tile([128, 1024], f32, tag="big", name="ps_v")
                    xts = {}

                    def vmm(cv):
                        # two 256-col accumulators per psum bank: only the
                        # first-issued zeroes the bank, the last closes it.
                        xv = xts.pop(cv)
                        for tl in range(4):
                            o = tl * 256
                            nc.tensor.matmul(
                                ps_v[:, o : o + 256],
                                xv[:, tl * 128 : (tl + 1) * 128],
                                wv_sb[:, cv * 256 : (cv + 1) * 256],
                                start=(cv == 0 and tl % 2 == 0),
                                stop=(cv == CT - 1 and tl % 2 == 1),
                            )

                    for c in range(CT):
                        xt = get_xt(b, j, c)
                        xts[c] = xt
                        for part in range(4):
                            dst = ps_q if part < 2 else ps_k
                            o = (part % 2) * 512
                            nc.tensor.matmul(
                                dst[:, o : o + 512],
                                wqk_sb[:, c * 512 + part * 128 : c * 512 + (part + 1) * 128],
                                xt[:, :],
                                start=(c == 0),
                                stop=(c == CT - 1),
                            )
                        if c >= 2:
                            vmm(c - 2)
                    if j + 1 < NJ:
                        for c in (0, 1):
                            stash[(b, j + 1, c)] = xt_dma(b, j + 1, c)
                    vmm(CT - 2)
                    vmm(CT - 1)

                    # drains: ACT takes q, DVE takes k then v (strided)
                    pc_q = pcpool.tile([128, 1024], bf16, tag="pc", name="pc_q")
                    pc_k = pcpool.tile([128, 1024], bf16, tag="pc", name="pc_k")
                    nc.scalar.copy(pc_q[:, :], ps_q[:, :])
                    nc.vector.tensor_copy(pc_k[:, :], ps_k[:, :])
                    base = j * 4 * TW
                    vdst = (
                        v_sb[:, base : base + 4 * TW]
                        .rearrange("p (t h x) -> p t h x", t=4, h=HPC)[:, :, :, 0:128]
                    )
                    vsrc = ps_v[:, :].rearrange("p (t h x) -> p t h x", t=4, h=HPC)
                    nc.vector.tensor_copy(vdst, vsrc)

                    # rope (bf16, DVE): [E|O] x [c|s] and [s|c]
                    blk = (b * NJ + j) * 1024
                    cs_t = cossin_sb[:, blk : blk + 1024]
                    sc_t = sincos_sb[:, blk : blk + 1024]
                    for name, pc_ in (("q", pc_q), ("k", pc_k)):
                        oe = ropool.tile([128, 512], bf16, tag="ro", name=f"{name}e_r")
                        oo = ropool.tile([128, 512], bf16, tag="ro", name=f"{name}o_r")
                        t12 = rtpool.tile([128, 1024], bf16, tag="rt", name="t12")
                        t34 = rtpool.tile([128, 1024], bf16, tag="rt", name="t34")
                        nc.vector.tensor_mul(t12[:, :], pc_[:, :], cs_t)
                        nc.vector.tensor_mul(t34[:, :], pc_[:, :], sc_t)
                        nc.vector.tensor_sub(oe[:, :], t12[:, 0:512], t12[:, 512:1024])
                        nc.vector.tensor_add(oo[:, :], t34[:, 0:512], t34[:, 512:1024])
                        dst = qhat if name == "q" else khat
                        js = slice(j * 512, (j + 1) * 512)
                        for h in range(HPC):
                            hs = slice(64 * h, 64 * h + 64)
                            nc.sync.dma_start(out=dst[h][0:64, js], in_=oe[hs, :])
                            nc.sync.dma_start(out=dst[h][64:128, js], in_=oo[hs, :])

                # ================= attention + wo phase =================
                yT = [ytpool.tile([128, T], bf16, tag="yt", name=f"yT{_h}") for _h in range(HPC)]

                def norm_dve(y_a, y_b):
                    ys = []
                    for tau in range(4):
                        dst = y_a if tau < 2 else y_b
                        off = (tau % 2) * 256
                        r = rpool.tile([128, 1], f32, tag="r", name="r")
                        nc.vector.reciprocal(r[:, :], dst[:, off + 128 : off + 129])
                        y_sb = ysbpool.tile([128, 128], bf16, tag="y", name="y_sb")
                        nc.vector.tensor_scalar_mul(
                            y_sb[:, :], dst[:, off : off + 128], r[:, 0:1]
                        )
                        ys.append(y_sb)
                    return ys

                def norm_pe(h, jj, ys):
                    for tau in range(4):
                        yt_ps = pbig.tile([128, 128], bf16, tag="big", name="yt_ps")
                        nc.tensor.transpose(yt_ps[:, :], ys[tau][:, :], ident[:, :])
                        g = 4 * jj + tau
                        nc.vector.tensor_copy(
                            yT[h][:, g * 128 : (g + 1) * 128], yt_ps[:, :]
                        )
                        emit_wo(1)

                def av(dst_pair, ex_ap, i, jj, p):
                    # AV matmuls for one k-tile: taus descending; the odd tau
                    # of each shared bank owns start (first issued) and stop
                    # (last finished).
                    y_a, y_b = dst_pair
                    for tau in range(3, max(0, p) - 1, -1):
                        dst = y_a if tau < 2 else y_b
                        off = (tau % 2) * 256
                        hi = tau % 2 == 1
                        nc.tensor.matmul(
                            dst[:, off : off + 129],
                            ex_ap[:, tau * 128 : (tau + 1) * 128],
                            v_sb[:, i * TW + h * VW : i * TW + h * VW + 129],
                            start=(i == 0 and hi),
                            stop=(i == 4 * jj + tau and hi),
                        )

                for jp in range(NJ // 2):
                    jlo, jhi = 2 * jp, 2 * jp + 1
                    nsc = 4 * jhi + 4
                    for h in range(HPC):
                        y_a = pacc.tile([128, 512], f32, tag="acc", name="y_a")
                        y_b = pacc.tile([128, 512], f32, tag="acc", name="y_b")
                        sc_tiles = {}
                        ex_tiles = {}

                        def issue_sc(i):
                            p_l = i - 4 * jlo
                            p_h = i - 4 * jhi
                            sc = pbig.tile([128, 1024], f32, tag="big", name="sc")
                            if i <= 4 * jlo + 3:
                                lo = max(0, p_l) * 128
                                nc.tensor.matmul(
                                    sc[:, lo:512],
                                    khat[h][:, i * 128 : (i + 1) * 128],
                                    qhat[h][:, jlo * 512 + lo : (jlo + 1) * 512],
                                    start=True,
                                    stop=True,
                                )
                            lo_h = 512 + max(0, p_h) * 128
                            nc.tensor.matmul(
                                sc[:, lo_h:1024],
                                khat[h][:, i * 128 : (i + 1) * 128],
                                qhat[h][:, jhi * 512 + lo_h - 512 : (jhi + 1) * 512],
                                start=True,
                                stop=True,
                            )
                            sc_tiles[i] = sc

                        issue_sc(0)
                        for i in range(nsc):
                            p_l = i - 4 * jlo
                            p_h = i - 4 * jhi
                            sc = sc_tiles.pop(i)
                            ex = expool.tile([128, 1024], bf16, tag="ex", name="ex")
                            ex_tiles[i] = ex
                            if i <= 4 * jlo + 3:
                                start = max(0, p_l) * 128
                            else:
                                start = 512 + max(0, p_h) * 128
                            nc.scalar.activation(
                                ex[:, start:1024], sc[:, start:1024], EXPF, scale=SCALE
                            )
                            if 0 <= p_l:
                                lo = p_l * 128
                                nc.gpsimd.tensor_mul(
                                    ex[:, lo : lo + 128], ex[:, lo : lo + 128], tri_sb[:, :]
                                )
                            if 0 <= p_h:
                                lo = 512 + p_h * 128
                                nc.gpsimd.tensor_mul(
                                    ex[:, lo : lo + 128], ex[:, lo : lo + 128], tri_sb[:, :]
                                )
                            if i + 1 < nsc:
                                issue_sc(i + 1)
                            emit_wo(1)
                            if i <= 4 * jlo + 3:
                                av((y_a, y_b), ex[:, 0:512], i, jlo, p_l)
                        # jlo done: DVE-normalize frees the acc slots, then the
                        # dependency-free jhi AV sweep keeps the PE hot while
                        # DVE/ACT catch up; jlo transposes after the sweep.
                        ys_lo = norm_dve(y_a, y_b)
                        y_a2 = pacc.tile([128, 512], f32, tag="acc", name="y_a2")
                        y_b2 = pacc.tile([128, 512], f32, tag="acc", name="y_b2")
                        for i in range(nsc):
                            av((y_a2, y_b2), ex_tiles.pop(i)[:, 512:1024], i, jhi, i - 4 * jhi)
                        norm_pe(h, jlo, ys_lo)
                        ys_hi = norm_dve(y_a2, y_b2)
                        emit_wo(2)
                        norm_pe(h, jhi, ys_hi)
                    # enqueue wo micro-ops for this pair (sprinkled into next)
                    for tt in range(8 * jp, 8 * jp + 8):
                        yo = yopool.tile([128, C], bf16, tag="yo", name="yo")
                        state = [0]
                        for obp in range(2):
                            wo_items.append((tt, obp, yo, state, yT, n0))
                if b + 1 < B:
                    for c in (0, 1):
                        stash[(b + 1, 0, c)] = xt_dma(b + 1, 0, c)
                    # carry 8 wo micro-ops into the next batch's first pair
                    # as PE filler for its otherwise filler-less i-loop
                    emit_wo(max(0, len(wo_items) - 8))
                else:
                    emit_wo(len(wo_items))
    nc.finalize()
    return nc


def _prep_inputs(x, w_qkv, w_o, rope_cos, rope_sin):
    import ml_dtypes

    bf = ml_dtypes.bfloat16
    xTh = np.ascontiguousarray(x.reshape(N, C).T).astype(bf)
    cosT = np.concatenate([rope_cos.T, rope_cos.T], 0)  # [128, T]
    sinT = np.concatenate([rope_sin.T, rope_sin.T], 0)
    cosF = np.tile(cosT, (1, B)).reshape(128, N // 512, 512)
    sinF = np.tile(sinT, (1, B)).reshape(128, N // 512, 512)
    cossin = np.stack([cosF, sinF], axis=2).reshape(128, 2 * N)
    sincos = np.stack([sinF, cosF], axis=2).reshape(128, 2 * N)

    r = np.arange(128)[:, None]
    c = np.arange(128)[None, :]
    tri = (c >= r).astype(np.float32)

    ev = np.arange(0, D, 2)
    od = np.arange(1, D, 2)
    in_maps = []
    for m in range(NCORES):
        h0, h1 = 2 * m, 2 * m + 1
        QE = np.concatenate([w_qkv[h0 * D + ev, :], w_qkv[h1 * D + ev, :]], 0).T
        QO = np.concatenate([w_qkv[h0 * D + od, :], w_qkv[h1 * D + od, :]], 0).T
        KE = np.concatenate([w_qkv[C + h0 * D + ev, :], w_qkv[C + h1 * D + ev, :]], 0).T
        KO = np.concatenate([w_qkv[C + h0 * D + od, :], w_qkv[C + h1 * D + od, :]], 0).T
        Wqk = np.concatenate([QE, QO, KE, KO], 1)  # [C, 512]
        wqk2 = Wqk.reshape(CT, 128, 512).transpose(1, 0, 2).reshape(128, CT * 512)
        Wv = w_qkv[2 * C + 2 * m * D : 2 * C + (2 * m + 2) * D, :].T  # [C, 256]
        wv2 = Wv.reshape(CT, 128, 256).transpose(1, 0, 2).reshape(128, CT * 256)
        Wo = w_o[:, 2 * m * D : (2 * m + 2) * D].T  # [256, C]
        wo2 = Wo.reshape(HPC, 128, C).transpose(1, 0, 2).reshape(128, HPC * C)
        in_maps.append(
            {
                "xT": xTh,
                "wqk2": np.ascontiguousarray(wqk2).astype(bf),
                "wv2": np.ascontiguousarray(wv2).astype(bf),
                "wo2": np.ascontiguousarray(wo2).astype(bf),
                "cossin": np.ascontiguousarray(cossin).astype(bf),
                "sincos": np.ascontiguousarray(sincos).astype(bf),
                "tri": np.ascontiguousarray(tri).astype(bf),
            }
        )
    return in_maps


def kernel(x, w_qkv, w_o, rope_cos, rope_sin, _trace=False):
    global _COMPILED
    x = np.asarray(x, dtype=np.float32)
    w_qkv = np.asarray(w_qkv, dtype=np.float32)
    w_o = np.asarray(w_o, dtype=np.float32)
    rope_cos = np.asarray(rope_cos, dtype=np.float32)
    rope_sin = np.asarray(rope_sin, dtype=np.float32)

    from concourse.bass_utils import run_bass_kernel_spmd

    if _COMPILED is None:
        _COMPILED = _build()
    nc = _COMPILED
    in_maps = _prep_inputs(x, w_qkv, w_o, rope_cos, rope_sin)
    res = run_bass_kernel_spmd(
        nc, in_maps, core_ids=list(range(NCORES)), trace=_trace
    )
    out = np.zeros((N, C), dtype=np.float32)
    for m in range(NCORES):
        out += res.results[m]["out_p"].astype(np.float32)
    kernel._last_results = res
    return out.reshape(B, T, C)
